# revision 28
# baseline (speedup 1.0000x reference)
"""NT-Xent contrastive loss on 8 Trainium2 NeuronCores (Bass/Tile), fp8.

Strategy (no collectives; ncfw collective latency floor ~85us):
  * Host casts embT to fp8e4 [2048, 8192] (sigma=1 fits e4m3) and W*64 to
    fp8e4; b*64 stays f32.  Slab cover: core c loads the 4 column-slabs
    S_c = {c, c+1, c+2, c+4} (mod 8) of embT (8.4 MB/core).  Every slab
    pair meets on some core (Z8 difference cover), so each distinct
    1024x1024 block of the 8192x8192 similarity matrix is computed once
    globally (the diff-4 block is deduped on host: cores 0-3 win).
  * Per core: head matmul in fp8 DoubleRow (K=256/instr), interleaved
    with the previous stage's sim units for PE density -> h' = 64h in
    psum -> DVE bias-add copy to bf16.  L2 norm: nsq' / 64 via a
    (1/64)-ones bf16 matmul, then r = exp(-0.5*ln(x)) on ACT (Exp/Ln
    pinned to the one activation table that holds both => single table
    load), broadcast down partitions with gpsimd partition_broadcast,
    t_on = h*r in fp8e4 (= 8 * normalized out).  norm(k) is emitted two
    units before the previous stage's exp stream drains, so the stage
    boundary costs no ACT idle time.
  * 5 sim blocks/core (diag + 4 pairs): one DoubleRow matmul per
    [128,1024] psum tile.  The diag block is symmetric, so only its
    upper-triangle tiles (columns >= 128*mb) are computed; a tril
    [128,128] -1e9 mask kills the diagonal and the within-block lower
    half pre-exp, and the missing lower-triangle row sums come back as
    column sums of the upper part (extra e5 colsum slot 4).  ACT
    exp(0.15625*x) with fused row-sum accum writes fp8e5 exp values;
    column sums via DoubleRow ones-matmuls over mb-pair-interleaved e5
    tiles at the end, each shipped to DRAM as soon as it completes.
  * pos: bf16 product of t_h slabs 0,3 + ones-matmul + r-scales (the
    1/64 ones make possim come out unscaled).  Host combine in fp64.
"""
import math
import numpy as np
import ml_dtypes

SLOTS = [(c, (c + 1) % 8, (c + 2) % 8, (c + 4) % 8) for c in range(8)]
# sim units: (stationary slot, moving slot, e5 colsum slot)
UNITS = [(0, 0, 4), (0, 1, 0), (0, 2, 1), (0, 3, 2), (1, 3, 3)]

_CACHE = {}


def _build():
    if "nc" in _CACHE:
        return _CACHE["nc"]
    import concourse.bacc as bacc
    import concourse.tile as tile
    import concourse.mybir as mybir

    F32 = mybir.dt.float32
    BF16 = mybir.dt.bfloat16
    E4 = mybir.dt.float8e4
    E5 = mybir.dt.float8e5
    AF = mybir.ActivationFunctionType
    ALU = mybir.AluOpType
    DR = mybir.MatmulPerfMode.DoubleRow

    nc = bacc.Bacc("TRN2", num_devices=8, debug=False)
    a_emb = nc.dram_tensor("embT8", [2048, 4096], E4, kind="ExternalInput").ap()
    a_W = nc.dram_tensor("W8", [2048, 256], E4, kind="ExternalInput").ap()
    a_b = nc.dram_tensor("b64", [256], F32, kind="ExternalInput").ap()
    a_o1 = nc.dram_tensor("onesbf", [128, 1], BF16, kind="ExternalInput").ap()
    a_o5 = nc.dram_tensor("ones5", [128, 256], E5, kind="ExternalInput").ap()
    a_mask = nc.dram_tensor("mask", [128, 128], F32, kind="ExternalInput").ap()
    o_rp = nc.dram_tensor("rowpart", [128, 40], F32, kind="ExternalOutput").ap()
    o_cp = nc.dram_tensor("colpart", [1, 5120], F32, kind="ExternalOutput").ap()
    o_ps = nc.dram_tensor("possim", [1, 1024], F32, kind="ExternalOutput").ap()

    with tile.TileContext(nc) as tc:
        with tc.tile_pool(name="sb", bufs=1) as sb, \
             tc.tile_pool(name="wk", bufs=2) as wk, \
             tc.tile_pool(name="hp", bufs=2, space="PSUM") as hp, \
             tc.tile_pool(name="simp", bufs=2, space="PSUM") as simp, \
             tc.tile_pool(name="smp", bufs=1, space="PSUM") as smp:

            # ---- persistent tiles + prologue DMAs, critical path first:
            # t_W, then stage-0 emb tiles, then the small constants, then
            # stages 1-3 (all on the sync queue; a second DMA queue measured
            # slower -- gpsimd dma issues block its own instruction stream).
            t_Wh = [sb.tile([128, 4, 2, 2, 128], E4, name=f"t_W{i}")
                    for i in range(2)]
            w_src = a_W.rearrange("(kk j p) (dh f) -> p kk j dh f",
                                  kk=8, j=2, p=128, dh=2, f=128)
            nc.sync.dma_start(t_Wh[0][:], w_src[:, 0:4])

            def tW(kk, dh):
                return t_Wh[kk // 4][:, kk % 4, :, dh, :]
            t_e8 = [[None] * 8 for _ in range(4)]
            def load_emb(k, kk, eng):
                t = sb.tile([128, 2, 1024], E4, name=f"t_e8_{k}_{kk}")
                esrc = a_emb[256 * kk:256 * (kk + 1),
                             1024 * k:1024 * (k + 1)]
                eng.dma_start(t[:], esrc.rearrange("(j p) s -> p j s",
                                                   j=2, p=128))
                t_e8[k][kk] = t
            load_emb(0, 0, nc.sync)
            nc.sync.dma_start(t_Wh[1][:], w_src[:, 4:8])
            for kk in range(1, 8):
                load_emb(0, kk, nc.sync)
            t_b = sb.tile([128, 2], F32, name="t_b")
            nc.sync.dma_start(t_b[:], a_b.rearrange("(dh p) -> p dh",
                                                    p=128))
            t_o1 = sb.tile([128, 1], BF16, name="t_o1")
            nc.sync.dma_start(t_o1[:], a_o1[:])
            t_o5 = sb.tile([128, 2, 128], E5, name="t_o5")
            nc.sync.dma_start(t_o5[:], a_o5.rearrange("p (j f) -> p j f",
                                                      j=2, f=128))
            t_mask = sb.tile([128, 128], F32, name="t_mask")
            nc.sync.dma_start(t_mask[:], a_mask[:])
            for kk in range(8):
                load_emb(1, kk, nc.sync)
            for kk in range(8):
                load_emb(2, kk, nc.sync)
            for kk in range(8):
                load_emb(3, kk, nc.sync)

            t_h = [sb.tile([128, 2, 1024], BF16, name=f"t_h{k}")
                   for k in range(4)]
            t_r_tiles = [None] * 4
            t_on = sb.tile([128, 2, 4, 1024], E4, name="t_on")
            t_e5 = sb.tile([128, 8, 5, 1024], E5, name="t_e5")
            rp_a = sb.tile([128, 32], F32, name="rp_a")
            rp_b = sb.tile([128, 8], F32, name="rp_b")
            cp_st = [sb.tile([1, 1024], F32, name=f"cp_st{i}")
                     for i in range(5)]
            ps_st = sb.tile([1, 1024], F32, name="ps_st")

            # zero the never-written lower-triangle region of the diag e5
            # slot while DVE is otherwise idle during the DMA phase
            for mb in range(1, 8):
                nc.vector.memset(t_e5[:, mb, 4, 0:128 * mb], 0)

            # PE warm-up: throwaway matmuls on a zeroed tile during the
            # DMA wait release the HAM clock gate before the head chains
            t_warm = sb.tile([128, 2, 512], E4, name="t_warm")
            nc.vector.memset(t_warm[:], 0)
            for i in range(4):
                Hw = hp.tile([128, 512], F32, name=f"Hw{i}", tag="H")
                nc.tensor.matmul(Hw[:], t_warm[:, :, 0:128], t_warm[:],
                                 start=True, stop=True, perf_mode=DR)

            def head_chain(k, dh, h):
                H = hp.tile([128, 512], F32, name=f"H{k}_{dh}_{h}", tag="H")
                for kk in range(8):
                    nc.tensor.matmul(
                        H[:], tW(kk, dh),
                        t_e8[k][kk][:, :, 512 * h:512 * (h + 1)],
                        start=(kk == 0), stop=(kk == 7), perf_mode=DR)
                nc.vector.tensor_scalar_add(
                    t_h[k][:, dh, 512 * h:512 * (h + 1)], H[:],
                    t_b[:, dh:dh + 1])

            nsq_t = [None] * 4
            nrm_t = [None] * 4

            def norm_q(k, dh, h):
                # one quarter of the norm pipeline, emitted right after
                # head chain (k, dh, h): the nsq column-half h aligns with
                # that chain's output rows, and once both dh contributions
                # of a half are in, its ln/exp/broadcast/scale tail runs
                # while the remaining chains still occupy the PE
                hs = np.s_[512 * h:512 * (h + 1)]
                t_sqq = wk.tile([128, 512], BF16, name="t_sqq", tag="sq")
                nc.vector.tensor_tensor(t_sqq[:], t_h[k][:, dh, hs],
                                        t_h[k][:, dh, hs], ALU.mult)
                if dh == 0 and h == 0:
                    # t_o1 holds 1/64, so nsq psum = nsq'/64 and
                    # exp(-0.5*ln(x)) = 8/sqrt(nsq')
                    nsq_t[k] = smp.tile([1, 1024], F32, name=f"nsq{k}",
                                        tag="sm")
                nc.tensor.matmul(nsq_t[k][0:1, hs], t_o1[:], t_sqq[:],
                                 start=(dh == 0), stop=(dh == 1),
                                 skip_group_check=True)
                if dh == 1 and h == 0:
                    nln = wk.tile([1, 1024], F32, name="nln", tag="nln")
                    t_rk = sb.tile([1, 1024], F32, name=f"t_r{k}")
                    t_r_tiles[k] = t_rk
                    r_bc = wk.tile([128, 1024], F32, name="r_bc", tag="rbc")
                    nrm_t[k] = (nln, t_rk, r_bc)
                if dh == 1:
                    nln, t_rk, r_bc = nrm_t[k]
                    nc.scalar.activation(nln[0:1, hs], nsq_t[k][0:1, hs],
                                         AF.Ln)
                    nc.scalar.activation(t_rk[0:1, hs], nln[0:1, hs],
                                         AF.Exp, scale=-0.5)
                    nc.gpsimd.partition_broadcast(r_bc[:, hs],
                                                  t_rk[0:1, hs])
                    for d2 in range(2):
                        nc.vector.tensor_tensor(t_on[:, d2, k, hs],
                                                t_h[k][:, d2, hs],
                                                r_bc[:, hs], ALU.mult)

            def unit(u, a, b, e5slot, mb):
                ps = simp.tile([128, 1024], F32, name="ps", tag="ps")
                lo = 128 * mb if a == b else 0
                # diag blocks are symmetric: compute only columns >= 128*mb
                # (upper triangle at tile granularity); the lower-triangle
                # row sums come back as column sums of the upper part
                for s0, s1 in ((lo, 512), (512, 1024)):
                    s0 = max(s0, lo)
                    if s1 <= s0:
                        continue
                    nc.tensor.matmul(ps[:, s0:s1],
                                     t_on[:, :, a, 128 * mb:128 * (mb + 1)],
                                     t_on[:, :, b, s0:s1],
                                     start=True, stop=True, perf_mode=DR)
                if a == b:
                    # tril -1e9 mask kills the diagonal and the
                    # within-block lower half (first 128 computed columns)
                    nc.vector.tensor_tensor(
                        ps[:, lo:lo + 128], ps[:, lo:lo + 128], t_mask[:],
                        ALU.add)
                acc = rp_a[:, u * 8 + mb:u * 8 + mb + 1] if u < 4 else \
                    rp_b[:, mb:mb + 1]
                nc.scalar.activation(t_e5[:, mb, e5slot, lo:1024],
                                     ps[:, lo:1024], AF.Exp, scale=0.15625,
                                     accum_out=acc)

            def stage_full(k):
                for dh in range(2):
                    for h in range(2):
                        head_chain(k, dh, h)
                        norm_q(k, dh, h)

            def colsums(ci):
                # column sums for e5 slot ci (DoubleRow over mb pairs),
                # shipped to DRAM as soon as both halves are staged
                for nh in range(2):
                    cs = hp.tile([128, 512], F32, name=f"cs{ci}_{nh}",
                                 tag="H")
                    for jj in range(4):
                        nc.tensor.matmul(
                            cs[:], t_o5[:],
                            t_e5[:, 2 * jj:2 * jj + 2, ci,
                                 512 * nh:512 * (nh + 1)],
                            start=(jj == 0), stop=(jj == 3), perf_mode=DR)
                    nc.vector.tensor_copy(
                        cp_st[ci][0:1, 512 * nh:512 * (nh + 1)], cs[0:1, :])
                nc.sync.dma_start(o_cp[0:1, 1024 * ci:1024 * (ci + 1)],
                                  cp_st[ci][:])

            def emit_pos():
                # pos: bf16 product of t_h slabs 0 and 3 + ones-matmul +
                # r-scales (1/64 ones make possim come out unscaled)
                t_pp = wk.tile([128, 2, 1024], BF16, name="t_pp", tag="sq")
                nc.vector.tensor_tensor(t_pp[:], t_h[0][:], t_h[3][:],
                                        ALU.mult)
                pr = smp.tile([1, 1024], F32, name="rawdot", tag="sm")
                for nh in range(2):
                    for dh in range(2):
                        nc.tensor.matmul(
                            pr[0:1, 512 * nh:512 * (nh + 1)], t_o1[:],
                            t_pp[:, dh, 512 * nh:512 * (nh + 1)],
                            start=(dh == 0), stop=(dh == 1))
                tmp = wk.tile([1, 1024], F32, name="ptmp", tag="nln")
                nc.vector.tensor_tensor(tmp[:], pr[:], t_r_tiles[0][:],
                                        ALU.mult)
                nc.vector.tensor_tensor(ps_st[:], tmp[:], t_r_tiles[3][:],
                                        ALU.mult)
                nc.sync.dma_start(o_ps, ps_st[:])

            def emit_unit(u, mb):
                unit(u, *UNITS[u][:2], UNITS[u][2], mb)

            stage_full(0)
            for k in range(1, 4):
                pu = k - 1
                emit_unit(pu, 0)
                emit_unit(pu, 1)
                head_chain(k, 0, 0)
                norm_q(k, 0, 0)
                emit_unit(pu, 2)
                emit_unit(pu, 3)
                head_chain(k, 0, 1)
                norm_q(k, 0, 1)
                emit_unit(pu, 4)
                head_chain(k, 1, 0)
                norm_q(k, 1, 0)
                emit_unit(pu, 5)
                head_chain(k, 1, 1)
                norm_q(k, 1, 1)
                emit_unit(pu, 6)
                emit_unit(pu, 7)
            for mb in range(8):
                emit_unit(3, mb)
                if mb == 0:
                    emit_pos()
                elif mb == 1:
                    colsums(0)
                elif mb == 5:
                    colsums(1)
            # rowpart for units 0-3 is complete here; ship it early
            nc.sync.dma_start(o_rp[:, 0:32], rp_a[:])
            cs3 = [None, None]
            for mb in range(8):
                emit_unit(4, mb)
                if mb == 2:
                    colsums(2)
                elif mb == 4:
                    colsums(4)
                elif mb == 5:
                    # start colsums(3) on the six u4 tiles already exp'd
                    # so only one mb-pair chunk remains after the last exp
                    for nh in range(2):
                        cs3[nh] = hp.tile([128, 512], F32,
                                          name=f"cs3_{nh}", tag="H")
                        for jj in range(3):
                            nc.tensor.matmul(
                                cs3[nh][:], t_o5[:],
                                t_e5[:, 2 * jj:2 * jj + 2, 3,
                                     512 * nh:512 * (nh + 1)],
                                start=(jj == 0), stop=False, perf_mode=DR)
            for nh in range(2):
                nc.tensor.matmul(
                    cs3[nh][:], t_o5[:],
                    t_e5[:, 6:8, 3, 512 * nh:512 * (nh + 1)],
                    start=False, stop=True, perf_mode=DR)
                nc.vector.tensor_copy(
                    cp_st[3][0:1, 512 * nh:512 * (nh + 1)], cs3[nh][0:1, :])
            nc.sync.dma_start(o_cp[0:1, 3072:4096], cp_st[3][:])
            nc.sync.dma_start(o_rp[:, 32:40], rp_b[:])

    # Keep Exp/Ln selectable only from the single table set that holds both,
    # so the compiler never ping-pongs ACT table loads between exp-only and
    # ln-only sets (1283ns per reload).  Entries stay in place so
    # act_func_set_id indices still match act_info.json.
    import concourse.bacc as bacc_mod
    orig_get = bacc_mod.get_activation_tables

    def _pinned_tables(arch):
        tabs = orig_get(arch)
        AFT = mybir.ActivationFunctionType
        both = [k for k, v in tabs.items() if AFT.Exp in v and AFT.Ln in v]
        if not both:
            return tabs
        keep = both[0]
        out = {}
        for k, v in tabs.items():
            if k == keep:
                out[k] = v
            else:
                out[k] = {f for f in v if f not in (AFT.Exp, AFT.Ln)}
        return out

    bacc_mod.get_activation_tables = _pinned_tables
    try:
        nc.compile()
    finally:
        bacc_mod.get_activation_tables = orig_get
    _CACHE["nc"] = nc
    return nc


def _host_inputs(embedded_data, W, b):
    E4np = ml_dtypes.float8_e4m3
    E5np = ml_dtypes.float8_e5m2
    emb = np.asarray(embedded_data, dtype=np.float32)
    embT8 = np.ascontiguousarray(emb.T).astype(E4np)      # [2048, 8192]
    W8 = (np.asarray(W, dtype=np.float32) * 64.0).astype(E4np)
    b64 = (np.asarray(b, dtype=np.float32) * 64.0).astype(np.float32)
    o1 = np.full((128, 1), 1.0 / 64.0, ml_dtypes.bfloat16)
    o5 = np.ones((128, 256), E5np)
    # -1e9 on the diagonal AND the within-block lower triangle: the diag
    # unit computes upper tiles [128mb:1024] whose first 128 columns hold
    # the block-diagonal square; masking its lower half (and diagonal)
    # makes acc+colsum count every off-diagonal element exactly once
    mask = np.where(np.tri(128, dtype=bool), -1e9, 0).astype(np.float32)
    in_maps = []
    for c in range(8):
        cols = np.concatenate(
            [embT8[:, 1024 * s:1024 * (s + 1)] for s in SLOTS[c]], axis=1)
        in_maps.append({"embT8": np.ascontiguousarray(cols), "W8": W8,
                        "b64": b64, "onesbf": o1, "ones5": o5, "mask": mask})
    return in_maps


def _combine(results):
    neg = np.zeros(8192, np.float64)
    pos = np.zeros(8192, np.float64)
    for c in range(8):
        S = SLOTS[c]
        rp = results[c]["rowpart"].astype(np.float64)     # [128, 40]
        cp = results[c]["colpart"].astype(np.float64).ravel()
        ps = results[c]["possim"].astype(np.float64)
        sl = [np.s_[1024 * s:1024 * (s + 1)] for s in S]
        for u, (astat, _, _) in enumerate(UNITS):
            if u == 3 and c >= 4:
                continue                                   # diff-4 dedup
            dst = 1024 * S[astat]
            for mb in range(8):
                neg[dst + 128 * mb:dst + 128 * (mb + 1)] += rp[:, 8 * u + mb]
        neg[sl[1]] += cp[0:1024]
        neg[sl[2]] += cp[1024:2048]
        if c < 4:
            neg[sl[3]] += cp[2048:3072]
        neg[sl[3]] += cp[3072:4096]
        # diag block is computed upper-triangle only; its column sums
        # supply the missing lower-triangle row sums
        neg[sl[0]] += cp[4096:5120]
        if c < 4:
            possim = ps.ravel()
            pos[sl[0]] = possim
            pos[sl[3]] = possim
    loss = -np.mean(10.0 * pos - np.log(neg))
    return np.float32(loss)


def run(embedded_data, W, b, trace=False):
    from concourse import bass_utils
    nc = _build()
    in_maps = _host_inputs(embedded_data, W, b)
    res = bass_utils.run_bass_kernel_spmd(nc, in_maps, core_ids=list(range(8)),
                                          trace=trace)
    return _combine(res.results), res


def kernel(embedded_data, W, b):
    loss, _ = run(embedded_data, W, b, trace=False)
    return np.asarray(loss, dtype=np.float32)


# revision 29
# speedup vs baseline: 1.0372x; 1.0372x over previous
"""NT-Xent contrastive loss on 8 Trainium2 NeuronCores (Bass/Tile), fp8.

Strategy (no collectives; ncfw collective latency floor ~85us):
  * Host casts embT to fp8e4 [2048, 8192] (sigma=1 fits e4m3) and W*64 to
    fp8e4; b*64 stays f32.  Slab cover: core c loads the 4 column-slabs
    S_c = {c, c+1, c+2, c+4} (mod 8) of embT (8.4 MB/core).  Every slab
    pair meets on some core (Z8 difference cover), so each distinct
    1024x1024 block of the 8192x8192 similarity matrix is computed once
    globally (the diff-4 block is deduped on host: cores 0-3 win).
  * Per core: head matmul in fp8 DoubleRow (K=256/instr), interleaved
    with the previous stage's sim units for PE density -> h' = 64h in
    psum -> DVE bias-add copy to bf16.  L2 norm: nsq' / 64 via a
    (1/64)-ones bf16 matmul, then r = exp(-0.5*ln(x)) on ACT (Exp/Ln
    pinned to the one activation table that holds both => single table
    load), broadcast down partitions with gpsimd partition_broadcast,
    t_on = h*r in fp8e4 (= 8 * normalized out).  norm(k) is emitted two
    units before the previous stage's exp stream drains, so the stage
    boundary costs no ACT idle time.
  * 5 sim blocks/core (diag + 4 pairs): one DoubleRow matmul per
    [128,1024] psum tile.  The diag block is symmetric, so only its
    upper-triangle tiles (columns >= 128*mb) are computed; a tril
    [128,128] -1e9 mask kills the diagonal and the within-block lower
    half pre-exp, and the missing lower-triangle row sums come back as
    column sums of the upper part (extra e5 colsum slot 4).  ACT
    exp(0.15625*x) with fused row-sum accum writes fp8e5 exp values;
    column sums via DoubleRow ones-matmuls over mb-pair-interleaved e5
    tiles at the end, each shipped to DRAM as soon as it completes.
  * pos: bf16 product of t_h slabs 0,3 + ones-matmul + r-scales (the
    1/64 ones make possim come out unscaled).  Host combine in fp64.
"""
import math
import numpy as np
import ml_dtypes

SLOTS = [(c, (c + 1) % 8, (c + 2) % 8, (c + 4) % 8) for c in range(8)]
# sim units: (stationary slot, moving slot, e5 colsum slot)
UNITS = [(0, 0, 4), (0, 1, 0), (0, 2, 1), (0, 3, 2), (1, 3, 3)]

_CACHE = {}


def _build():
    if "nc" in _CACHE:
        return _CACHE["nc"]
    import concourse.bacc as bacc
    import concourse.tile as tile
    import concourse.mybir as mybir

    F32 = mybir.dt.float32
    BF16 = mybir.dt.bfloat16
    E4 = mybir.dt.float8e4
    E5 = mybir.dt.float8e5
    AF = mybir.ActivationFunctionType
    ALU = mybir.AluOpType
    DR = mybir.MatmulPerfMode.DoubleRow

    nc = bacc.Bacc("TRN2", num_devices=8, debug=False)
    a_emb = nc.dram_tensor("embT8", [2048, 4096], E4, kind="ExternalInput").ap()
    a_W = nc.dram_tensor("W8", [2048, 256], E4, kind="ExternalInput").ap()
    a_b = nc.dram_tensor("b64", [256], F32, kind="ExternalInput").ap()
    a_o1 = nc.dram_tensor("onesbf", [128, 1], BF16, kind="ExternalInput").ap()
    a_o5 = nc.dram_tensor("ones5", [128, 256], E5, kind="ExternalInput").ap()
    a_mask = nc.dram_tensor("mask", [128, 128], F32, kind="ExternalInput").ap()
    o_rp = nc.dram_tensor("rowpart", [128, 40], F32, kind="ExternalOutput").ap()
    o_cp = nc.dram_tensor("colpart", [1, 5120], F32, kind="ExternalOutput").ap()
    o_ps = nc.dram_tensor("possim", [1, 1024], F32, kind="ExternalOutput").ap()

    with tile.TileContext(nc) as tc:
        with tc.tile_pool(name="sb", bufs=1) as sb, \
             tc.tile_pool(name="wk", bufs=2) as wk, \
             tc.tile_pool(name="hp", bufs=2, space="PSUM") as hp, \
             tc.tile_pool(name="simp", bufs=2, space="PSUM") as simp, \
             tc.tile_pool(name="smp", bufs=1, space="PSUM") as smp:

            # ---- persistent tiles + prologue DMAs, critical path first:
            # t_W, then stage-0 emb tiles, then the small constants, then
            # stages 1-3 (all on the sync queue; a second DMA queue measured
            # slower -- gpsimd dma issues block its own instruction stream).
            t_Wh = [sb.tile([128, 4, 2, 2, 128], E4, name=f"t_W{i}")
                    for i in range(2)]
            w_src = a_W.rearrange("(kk j p) (dh f) -> p kk j dh f",
                                  kk=8, j=2, p=128, dh=2, f=128)
            nc.sync.dma_start(t_Wh[0][:], w_src[:, 0:4])

            def tW(kk, dh):
                return t_Wh[kk // 4][:, kk % 4, :, dh, :]
            t_e8 = [[None] * 8 for _ in range(4)]
            def load_emb(k, kk, eng):
                t = sb.tile([128, 2, 1024], E4, name=f"t_e8_{k}_{kk}")
                esrc = a_emb[256 * kk:256 * (kk + 1),
                             1024 * k:1024 * (k + 1)]
                eng.dma_start(t[:], esrc.rearrange("(j p) s -> p j s",
                                                   j=2, p=128))
                t_e8[k][kk] = t
            load_emb(0, 0, nc.sync)
            nc.sync.dma_start(t_Wh[1][:], w_src[:, 4:8])
            for kk in range(1, 8):
                load_emb(0, kk, nc.sync)
            t_b = sb.tile([128, 2], F32, name="t_b")
            nc.sync.dma_start(t_b[:], a_b.rearrange("(dh p) -> p dh",
                                                    p=128))
            t_o1 = sb.tile([128, 1], BF16, name="t_o1")
            nc.sync.dma_start(t_o1[:], a_o1[:])
            t_o5 = sb.tile([128, 2, 128], E5, name="t_o5")
            nc.sync.dma_start(t_o5[:], a_o5.rearrange("p (j f) -> p j f",
                                                      j=2, f=128))
            t_mask = sb.tile([128, 128], F32, name="t_mask")
            nc.sync.dma_start(t_mask[:], a_mask[:])
            for kk in range(8):
                load_emb(1, kk, nc.sync)
            for kk in range(8):
                load_emb(2, kk, nc.sync)
            for kk in range(8):
                load_emb(3, kk, nc.sync)

            t_h = [sb.tile([128, 2, 1024], BF16, name=f"t_h{k}")
                   for k in range(4)]
            t_r_tiles = [None] * 4
            t_on = sb.tile([128, 2, 4, 1024], E4, name="t_on")
            t_e5 = sb.tile([128, 8, 5, 1024], E5, name="t_e5")
            rp_a = sb.tile([128, 32], F32, name="rp_a")
            rp_b = sb.tile([128, 8], F32, name="rp_b")
            cp_st = [sb.tile([1, 1024], F32, name=f"cp_st{i}")
                     for i in range(5)]
            ps_st = sb.tile([1, 1024], F32, name="ps_st")

            # zero the never-written lower-triangle region of the diag e5
            # slot while DVE is otherwise idle during the DMA phase
            for mb in range(1, 8):
                nc.vector.memset(t_e5[:, mb, 4, 0:128 * mb], 0)

            # PE warm-up: throwaway matmuls on a zeroed tile during the
            # DMA wait release the HAM clock gate before the head chains
            t_warm = sb.tile([128, 2, 512], E4, name="t_warm")
            nc.vector.memset(t_warm[:], 0)
            for i in range(4):
                Hw = hp.tile([128, 512], F32, name=f"Hw{i}", tag="H")
                nc.tensor.matmul(Hw[:], t_warm[:, :, 0:128], t_warm[:],
                                 start=True, stop=True, perf_mode=DR)

            def head_chain(k, dh, h):
                H = hp.tile([128, 512], F32, name=f"H{k}_{dh}_{h}", tag="H")
                for kk in range(8):
                    nc.tensor.matmul(
                        H[:], tW(kk, dh),
                        t_e8[k][kk][:, :, 512 * h:512 * (h + 1)],
                        start=(kk == 0), stop=(kk == 7), perf_mode=DR)
                nc.vector.tensor_scalar_add(
                    t_h[k][:, dh, 512 * h:512 * (h + 1)], H[:],
                    t_b[:, dh:dh + 1])

            nsq_t = [None] * 4

            def norm_a(k):
                # dh=0 half of the norm reduction, emitted right after the
                # dh=0 head chains so it hides under the dh=1 chains
                t_sq0 = wk.tile([128, 1024], BF16, name="t_sq0", tag="sq")
                nc.vector.tensor_tensor(t_sq0[:], t_h[k][:, 0, :],
                                        t_h[k][:, 0, :], ALU.mult)
                # t_o1 holds 1/64, so nsq psum = nsq'/64 and
                # exp(-0.5*ln(x)) = 8/sqrt(nsq') -- no activation bias needed
                nsq = smp.tile([1, 1024], F32, name=f"nsq{k}", tag="sm")
                nsq_t[k] = nsq
                for nh in range(2):
                    nc.tensor.matmul(nsq[0:1, 512 * nh:512 * (nh + 1)],
                                     t_o1[:],
                                     t_sq0[:, 512 * nh:512 * (nh + 1)],
                                     start=True, stop=False,
                                     skip_group_check=True)

            def norm_b(k):
                # dh=1 half + per-512-half ln/exp/broadcast/scale pipeline:
                # the first unit matmul only needs the first t_on half, so
                # the exposed boundary latency is roughly halved
                t_sq1 = wk.tile([128, 1024], BF16, name="t_sq1", tag="sq")
                nc.vector.tensor_tensor(t_sq1[:], t_h[k][:, 1, :],
                                        t_h[k][:, 1, :], ALU.mult)
                nsq = nsq_t[k]
                for nh in range(2):
                    nc.tensor.matmul(nsq[0:1, 512 * nh:512 * (nh + 1)],
                                     t_o1[:],
                                     t_sq1[:, 512 * nh:512 * (nh + 1)],
                                     start=False, stop=True,
                                     skip_group_check=True)
                nln = wk.tile([1, 1024], F32, name="nln", tag="nln")
                t_rk = sb.tile([1, 1024], F32, name=f"t_r{k}")
                t_r_tiles[k] = t_rk
                r_bc = wk.tile([128, 1024], F32, name="r_bc", tag="rbc")
                for nh in range(2):
                    hs = np.s_[512 * nh:512 * (nh + 1)]
                    nc.scalar.activation(nln[0:1, hs], nsq[0:1, hs], AF.Ln)
                    nc.scalar.activation(t_rk[0:1, hs], nln[0:1, hs],
                                         AF.Exp, scale=-0.5)
                    nc.gpsimd.partition_broadcast(r_bc[:, hs],
                                                  t_rk[0:1, hs])
                    for dh in range(2):
                        nc.vector.tensor_tensor(t_on[:, dh, k, hs],
                                                t_h[k][:, dh, hs],
                                                r_bc[:, hs], ALU.mult)

            def unit(u, a, b, e5slot, mb):
                ps = simp.tile([128, 1024], F32, name="ps", tag="ps")
                lo = 128 * mb if a == b else 0
                # diag blocks are symmetric: compute only columns >= 128*mb
                # (upper triangle at tile granularity); the lower-triangle
                # row sums come back as column sums of the upper part
                for s0, s1 in ((lo, 512), (512, 1024)):
                    s0 = max(s0, lo)
                    if s1 <= s0:
                        continue
                    nc.tensor.matmul(ps[:, s0:s1],
                                     t_on[:, :, a, 128 * mb:128 * (mb + 1)],
                                     t_on[:, :, b, s0:s1],
                                     start=True, stop=True, perf_mode=DR)
                if a == b:
                    # tril -1e9 mask kills the diagonal and the
                    # within-block lower half (first 128 computed columns)
                    nc.vector.tensor_tensor(
                        ps[:, lo:lo + 128], ps[:, lo:lo + 128], t_mask[:],
                        ALU.add)
                acc = rp_a[:, u * 8 + mb:u * 8 + mb + 1] if u < 4 else \
                    rp_b[:, mb:mb + 1]
                nc.scalar.activation(t_e5[:, mb, e5slot, lo:1024],
                                     ps[:, lo:1024], AF.Exp, scale=0.15625,
                                     accum_out=acc)

            def stage_full(k):
                head_chain(k, 0, 0)
                head_chain(k, 0, 1)
                norm_a(k)
                head_chain(k, 1, 0)
                head_chain(k, 1, 1)
                norm_b(k)

            def colsums(ci):
                # column sums for e5 slot ci (DoubleRow over mb pairs),
                # shipped to DRAM as soon as both halves are staged
                for nh in range(2):
                    cs = hp.tile([128, 512], F32, name=f"cs{ci}_{nh}",
                                 tag="H")
                    for jj in range(4):
                        nc.tensor.matmul(
                            cs[:], t_o5[:],
                            t_e5[:, 2 * jj:2 * jj + 2, ci,
                                 512 * nh:512 * (nh + 1)],
                            start=(jj == 0), stop=(jj == 3), perf_mode=DR)
                    nc.vector.tensor_copy(
                        cp_st[ci][0:1, 512 * nh:512 * (nh + 1)], cs[0:1, :])
                nc.sync.dma_start(o_cp[0:1, 1024 * ci:1024 * (ci + 1)],
                                  cp_st[ci][:])

            def emit_pos():
                # pos: bf16 product of t_h slabs 0 and 3 + ones-matmul +
                # r-scales (1/64 ones make possim come out unscaled)
                t_pp = wk.tile([128, 2, 1024], BF16, name="t_pp", tag="sq")
                nc.vector.tensor_tensor(t_pp[:], t_h[0][:], t_h[3][:],
                                        ALU.mult)
                pr = smp.tile([1, 1024], F32, name="rawdot", tag="sm")
                for nh in range(2):
                    for dh in range(2):
                        nc.tensor.matmul(
                            pr[0:1, 512 * nh:512 * (nh + 1)], t_o1[:],
                            t_pp[:, dh, 512 * nh:512 * (nh + 1)],
                            start=(dh == 0), stop=(dh == 1))
                tmp = wk.tile([1, 1024], F32, name="ptmp", tag="nln")
                nc.vector.tensor_tensor(tmp[:], pr[:], t_r_tiles[0][:],
                                        ALU.mult)
                nc.vector.tensor_tensor(ps_st[:], tmp[:], t_r_tiles[3][:],
                                        ALU.mult)
                nc.sync.dma_start(o_ps, ps_st[:])

            def emit_unit(u, mb):
                unit(u, *UNITS[u][:2], UNITS[u][2], mb)

            stage_full(0)
            for k in range(1, 4):
                pu = k - 1
                emit_unit(pu, 0)
                emit_unit(pu, 1)
                head_chain(k, 0, 0)
                emit_unit(pu, 2)
                emit_unit(pu, 3)
                head_chain(k, 0, 1)
                norm_a(k)
                emit_unit(pu, 4)
                head_chain(k, 1, 0)
                emit_unit(pu, 5)
                head_chain(k, 1, 1)
                # norm_b before the last two units: its ln/exp halves slot
                # into the exp stream and the broadcast/multiply pipeline
                # overlaps exps 6-7, so U_k's first psim is ready the
                # moment exp 7 drains
                norm_b(k)
                emit_unit(pu, 6)
                emit_unit(pu, 7)
            for mb in range(8):
                emit_unit(3, mb)
                if mb == 0:
                    emit_pos()
                elif mb == 1:
                    colsums(0)
                elif mb == 5:
                    colsums(1)
            # rowpart for units 0-3 is complete here; ship it early
            nc.sync.dma_start(o_rp[:, 0:32], rp_a[:])
            cs3 = [None, None]
            for mb in range(8):
                emit_unit(4, mb)
                if mb == 2:
                    colsums(2)
                elif mb == 4:
                    colsums(4)
                elif mb == 5:
                    # start colsums(3) on the six u4 tiles already exp'd
                    # so only one mb-pair chunk remains after the last exp
                    for nh in range(2):
                        cs3[nh] = hp.tile([128, 512], F32,
                                          name=f"cs3_{nh}", tag="H")
                        for jj in range(3):
                            nc.tensor.matmul(
                                cs3[nh][:], t_o5[:],
                                t_e5[:, 2 * jj:2 * jj + 2, 3,
                                     512 * nh:512 * (nh + 1)],
                                start=(jj == 0), stop=False, perf_mode=DR)
            for nh in range(2):
                nc.tensor.matmul(
                    cs3[nh][:], t_o5[:],
                    t_e5[:, 6:8, 3, 512 * nh:512 * (nh + 1)],
                    start=False, stop=True, perf_mode=DR)
                nc.vector.tensor_copy(
                    cp_st[3][0:1, 512 * nh:512 * (nh + 1)], cs3[nh][0:1, :])
            nc.sync.dma_start(o_cp[0:1, 3072:4096], cp_st[3][:])
            nc.sync.dma_start(o_rp[:, 32:40], rp_b[:])

    # Keep Exp/Ln selectable only from the single table set that holds both,
    # so the compiler never ping-pongs ACT table loads between exp-only and
    # ln-only sets (1283ns per reload).  Entries stay in place so
    # act_func_set_id indices still match act_info.json.
    import concourse.bacc as bacc_mod
    orig_get = bacc_mod.get_activation_tables

    def _pinned_tables(arch):
        tabs = orig_get(arch)
        AFT = mybir.ActivationFunctionType
        both = [k for k, v in tabs.items() if AFT.Exp in v and AFT.Ln in v]
        if not both:
            return tabs
        keep = both[0]
        out = {}
        for k, v in tabs.items():
            if k == keep:
                out[k] = v
            else:
                out[k] = {f for f in v if f not in (AFT.Exp, AFT.Ln)}
        return out

    bacc_mod.get_activation_tables = _pinned_tables
    try:
        nc.compile()
    finally:
        bacc_mod.get_activation_tables = orig_get
    _CACHE["nc"] = nc
    return nc


def _host_inputs(embedded_data, W, b):
    E4np = ml_dtypes.float8_e4m3
    E5np = ml_dtypes.float8_e5m2
    emb = np.asarray(embedded_data, dtype=np.float32)
    embT8 = np.ascontiguousarray(emb.T).astype(E4np)      # [2048, 8192]
    W8 = (np.asarray(W, dtype=np.float32) * 64.0).astype(E4np)
    b64 = (np.asarray(b, dtype=np.float32) * 64.0).astype(np.float32)
    o1 = np.full((128, 1), 1.0 / 64.0, ml_dtypes.bfloat16)
    o5 = np.ones((128, 256), E5np)
    # -1e9 on the diagonal AND the within-block lower triangle: the diag
    # unit computes upper tiles [128mb:1024] whose first 128 columns hold
    # the block-diagonal square; masking its lower half (and diagonal)
    # makes acc+colsum count every off-diagonal element exactly once
    mask = np.where(np.tri(128, dtype=bool), -1e9, 0).astype(np.float32)
    in_maps = []
    for c in range(8):
        cols = np.concatenate(
            [embT8[:, 1024 * s:1024 * (s + 1)] for s in SLOTS[c]], axis=1)
        in_maps.append({"embT8": np.ascontiguousarray(cols), "W8": W8,
                        "b64": b64, "onesbf": o1, "ones5": o5, "mask": mask})
    return in_maps


def _combine(results):
    neg = np.zeros(8192, np.float64)
    pos = np.zeros(8192, np.float64)
    for c in range(8):
        S = SLOTS[c]
        rp = results[c]["rowpart"].astype(np.float64)     # [128, 40]
        cp = results[c]["colpart"].astype(np.float64).ravel()
        ps = results[c]["possim"].astype(np.float64)
        sl = [np.s_[1024 * s:1024 * (s + 1)] for s in S]
        for u, (astat, _, _) in enumerate(UNITS):
            if u == 3 and c >= 4:
                continue                                   # diff-4 dedup
            dst = 1024 * S[astat]
            for mb in range(8):
                neg[dst + 128 * mb:dst + 128 * (mb + 1)] += rp[:, 8 * u + mb]
        neg[sl[1]] += cp[0:1024]
        neg[sl[2]] += cp[1024:2048]
        if c < 4:
            neg[sl[3]] += cp[2048:3072]
        neg[sl[3]] += cp[3072:4096]
        # diag block is computed upper-triangle only; its column sums
        # supply the missing lower-triangle row sums
        neg[sl[0]] += cp[4096:5120]
        if c < 4:
            possim = ps.ravel()
            pos[sl[0]] = possim
            pos[sl[3]] = possim
    loss = -np.mean(10.0 * pos - np.log(neg))
    return np.float32(loss)


def run(embedded_data, W, b, trace=False):
    from concourse import bass_utils
    nc = _build()
    in_maps = _host_inputs(embedded_data, W, b)
    res = bass_utils.run_bass_kernel_spmd(nc, in_maps, core_ids=list(range(8)),
                                          trace=trace)
    return _combine(res.results), res


def kernel(embedded_data, W, b):
    loss, _ = run(embedded_data, W, b, trace=False)
    return np.asarray(loss, dtype=np.float32)
